# revision 41
# baseline (speedup 1.0000x reference)
"""2D DCT [8,32,256,256] on 8 TRN2 NeuronCores — raw Bass (no Tile).

Math: the reference's FFT-mirror trick is exactly the linear map
    dct1d(x)[k] = (1/L) * sum_m x[m] * cos(pi*k*(m+0.5)/L)
so with A[m,k] = cos(pi*k*(m+0.5)/L)/L the 2D DCT per [256,256] slice is
    out = A^T @ X @ A = (X^T A)^T A
i.e. two chained TensorEngine matmuls with NO transposes:
    V  = matmul(lhsT=X, rhs=A)   # V = X^T A   ([w, j] in PSUM)
    out= matmul(lhsT=V, rhs=A)   # V^T A = A^T X A  ([h', w'] in PSUM)

Sharding: fully data-parallel over batch — core b takes ip[b] (32
independent slices). bf16 staging in a [128, units, 2, 256] host layout
(contiguous per partition; unit 0 is the DCT matrix itself, units 1..32
the slices), f32 PSUM accumulation.

Raw-Bass engine plan (the Tile framework's entry/exit overhead and
per-instruction wait splitting cost ~8-10us here):
  SP (sync)  : one HWDGE ring for everything — graduated in-chunks
               (unit 0 rides with the first), then the out-chunks
               (issue stalls on copy sems; each DIRECT2D issue costs
               ~0.6-1.2us of sequencer time which conveniently paces
               the flood so other engines' instruction fetches aren't
               starved), final wait on out completions.
  PE         : 32 warm-up matmuls on garbage SBUF during the DMA head
               (HAM hits K=8/8 right as real data lands), then a
               software-pipelined stream S1(0..3), [S1(s), S2(s-4)],
               S2 tail; one pe_sem inc per 4-matmul stage.
  DVE / ACT  : whole-bank PSUM->SBUF evictions, one per stage,
               alternating engines per slice, each engine's stream
               sorted by pe_sem dependency (no head-of-line waits).
"""

import numpy as np

import concourse.bacc as bacc
import concourse.bass as bass
import concourse.mybir as mybir
from concourse.bass_utils import run_bass_kernel_spmd

N_CORES = 8
C = 32                    # slices per core (channel dim; batch is sharded)
L = 256                   # DCT length
BF16 = mybir.dt.bfloat16
F32 = mybir.dt.float32
NP_BF16 = mybir.dt.np(mybir.dt.bfloat16)

# In-chunks in UNITS of the staged tensor (unit 0 = DCT matrix,
# unit u = slice u-1). Chunk 0 (A + slice 0) goes via gpsimd SWDGE,
# whose stream starts ~1.5us before the sync ring's first completion;
# the rest are graduated on the sync HWDGE ring.
IN_CHUNKS = [2, 1, 2, 4, 6, 6, 6, 6]
OUT_CHUNKS = [6, 6, 6, 6, 4, 2]           # slices per sync-ring out-DMA
TAIL_OUT = (30, 32)        # final slices: ACT issues one merged out-DMA
N_WARM = 17               # HAM warm-up matmuls during the DMA head
PS_RV = 4                 # vp PSUM ring depth (banks)
PS_RO = 3                 # op PSUM ring depth (banks)
VS_R = 6                  # vs SBUF ring depth
LOOKAHEAD = PS_RV         # S2(s) issues LOOKAHEAD stages after S1(s)


def _dct_matrix() -> np.ndarray:
    m = np.arange(L, dtype=np.float64)
    k = np.arange(L, dtype=np.float64)
    a = np.cos(np.pi * np.outer(m + 0.5, k) / L) / L
    a = a.astype(np.float32).astype(NP_BF16)
    # pack for SBUF: [p, ki, w] with row ki*128+p on partition p
    return np.ascontiguousarray(a.reshape(2, 128, L).transpose(1, 0, 2))


def _chunk_of_slice(s):
    """Chunk index covering slice s (= unit s+1)."""
    u = s + 1
    c0 = 0
    for ci, n in enumerate(IN_CHUNKS):
        if u < c0 + n:
            return ci
        c0 += n
    raise AssertionError


def _pe_schedule():
    order = []
    for s in range(C):
        order.append(("S1", s))
        if s >= LOOKAHEAD:
            order.append(("S2", s - LOOKAHEAD))
    for s in range(C - LOOKAHEAD, C):
        order.append(("S2", s))
    pe_count = {st: i + 1 for i, st in enumerate(order)}
    return order, pe_count


def _copy_plan(pe_count):
    """vs_copy(s) dep: S1(s); os_copy(s) dep: S2(s). Alternate engines by
    slice parity (the tail slices' os copies pinned to ACT so it can
    issue their out-DMAs inline); per-engine streams sorted by dep."""
    streams = {"dve": [], "act": []}
    for s in range(C):
        streams["dve" if s % 2 == 0 else "act"].append((pe_count[("S1", s)], "vs", s))
        streams["act" if s % 2 == 0 else "dve"].append((pe_count[("S2", s)], "os", s))
    pos = {}
    for eng, evs in streams.items():
        evs.sort()
        for i, (dep, kind, s) in enumerate(evs):
            pos[(kind, s)] = (eng, i + 1, dep)
    return streams, pos


def _build(sim: bool = False) -> bass.Bass:
    nc = bacc.Bacc()
    x = nc.declare_dram_parameter("x", [128, C + 1, 2, L], BF16, isOutput=False)
    out = nc.declare_dram_parameter("out", [128, C, 2, L], BF16, isOutput=True)

    order, pe_count = _pe_schedule()
    streams, pos = _copy_plan(pe_count)

    from contextlib import ExitStack

    ctx = ExitStack()
    with ctx:
        warm_sb = ctx.enter_context(nc.sbuf_tensor([128, 128], BF16))
        xs = ctx.enter_context(nc.sbuf_tensor([128, C + 1, 2, L], BF16))
        vs = ctx.enter_context(nc.sbuf_tensor([128, VS_R, 2, L], BF16))
        os_ = ctx.enter_context(nc.sbuf_tensor([128, C, 2, L], BF16))
        vp = ctx.enter_context(nc.psum_tensor([128, PS_RV, 2, L], F32))
        op = ctx.enter_context(nc.psum_tensor([128, PS_RO, 2, L], F32))
        warm_ps = ctx.enter_context(nc.psum_tensor([128, 128], F32))

        in_sems = [
            ctx.enter_context(nc.semaphore(f"in_sem{i}"))
            for i in range(len(IN_CHUNKS))
        ]
        pe_sem = ctx.enter_context(nc.semaphore("pe_sem"))
        dve_sem = ctx.enter_context(nc.semaphore("dve_sem"))
        act_sem = ctx.enter_context(nc.semaphore("act_sem"))
        out_sem = ctx.enter_context(nc.semaphore("out_sem"))
        warm_sem = ctx.enter_context(nc.semaphore("warm_sem"))
        sem_of = {"dve": dve_sem, "act": act_sem}

        block = ctx.enter_context(nc.Block(no_gpsimd_drain=True))

        @block.gpsimd
        def _(eng):
            n0 = IN_CHUNKS[0]
            eng.dma_start(xs[:, 0:n0, :, :], x[:, 0:n0, :, :]).then_inc(
                in_sems[0], 16
            )

        @block.sync
        def _(eng):
            u0 = IN_CHUNKS[0]
            for ci, n in enumerate(IN_CHUNKS[1:], start=1):
                eng.dma_start(
                    xs[:, u0 : u0 + n, :, :], x[:, u0 : u0 + n, :, :]
                ).then_inc(in_sems[ci], 16)
                u0 += n
            c0 = 0
            for n in OUT_CHUNKS:
                for eng_name in ("dve", "act"):
                    need = max(
                        (
                            pos[("os", s)][1]
                            for s in range(c0, c0 + n)
                            if pos[("os", s)][0] == eng_name
                        ),
                        default=0,
                    )
                    if need:
                        eng.wait_ge(sem_of[eng_name], need)
                eng.dma_start(
                    out[:, c0 : c0 + n, :, :], os_[:, c0 : c0 + n, :, :]
                ).then_inc(out_sem, 16)
                c0 += n
            eng.wait_ge(out_sem, 16 * (len(OUT_CHUNKS) + 1))

        @block.tensor
        def _(eng):
            if sim:
                # CoreSim rejects reads of uninitialized SBUF; on HW the
                # warm-up matmuls happily consume garbage.
                eng.wait_ge(warm_sem, 1)
            for _ in range(N_WARM):
                nc.tensor.matmul(
                    warm_ps[:], warm_sb[:], warm_sb[:], start=True, stop=True
                )
            seen_chunks = set()
            for kind, s in order:
                if kind == "S1":
                    ci = _chunk_of_slice(s)
                    if ci not in seen_chunks:
                        seen_chunks.add(ci)
                        eng.wait_ge(in_sems[ci], 16)
                    if s >= PS_RV:
                        # vp ring slot reuse: vs_copy(s-PS_RV) done
                        e, p, _ = pos[("vs", s - PS_RV)]
                        eng.wait_ge(sem_of[e], p)
                    r = s % PS_RV
                    for mi in range(2):
                        for ki in range(2):
                            mm = nc.tensor.matmul(
                                vp[:, r, mi, :],
                                xs[:, s + 1, ki, mi * 128 : (mi + 1) * 128],
                                xs[:, 0, ki, :],
                                start=(ki == 0),
                                stop=(ki == 1),
                            )
                    mm.then_inc(pe_sem, 1)
                else:
                    e, p, _ = pos[("vs", s)]          # vs(s) staged
                    eng.wait_ge(sem_of[e], p)
                    if s >= PS_RO:
                        # op ring slot reuse: os_copy(s-PS_RO) done
                        e, p, _ = pos[("os", s - PS_RO)]
                        eng.wait_ge(sem_of[e], p)
                    r = s % PS_RO
                    for ji in range(2):
                        for wi in range(2):
                            mm = nc.tensor.matmul(
                                op[:, r, ji, :],
                                vs[:, s % VS_R, wi, ji * 128 : (ji + 1) * 128],
                                xs[:, 0, wi, :],
                                start=(wi == 0),
                                stop=(wi == 1),
                            )
                    mm.then_inc(pe_sem, 1)

        def copy_stream(eng_name):
            def body(eng):
                copy = (
                    nc.vector.tensor_copy if eng_name == "dve" else nc.scalar.copy
                )
                if eng_name == "dve" and sim:
                    nc.vector.memset(warm_sb[:], 0.0).then_inc(warm_sem, 1)
                for dep, kind, s in streams[eng_name]:
                    eng.wait_ge(pe_sem, dep)
                    if kind == "vs":
                        copy(vs[:, s % VS_R, :, :], vp[:, s % PS_RV, :, :]).then_inc(
                            sem_of[eng_name], 1
                        )
                    else:
                        copy(os_[:, s, :, :], op[:, s % PS_RO, :, :]).then_inc(
                            sem_of[eng_name], 1
                        )
                if eng_name == "act":
                    # merged tail out-DMA: both final os copies done
                    # (own one by own-sem, the other engine's by sem)
                    lo, hi = TAIL_OUT
                    for s in range(lo, hi):
                        e, p, _ = pos[("os", s)]
                        eng.wait_ge(sem_of[e], p)
                    eng.dma_start(
                        out[:, lo:hi, :, :], os_[:, lo:hi, :, :]
                    ).then_inc(out_sem, 16)
            return body

        block.vector(copy_stream("dve"))
        block.scalar(copy_stream("act"))

    nc.compile()
    return nc


_NC_CACHE: bass.Bass | None = None


def _get_nc() -> bass.Bass:
    global _NC_CACHE
    if _NC_CACHE is None:
        _NC_CACHE = _build()
    return _NC_CACHE


def _make_in_maps(ip: np.ndarray) -> list[dict[str, np.ndarray]]:
    a = _dct_matrix()[:, None, :, :]                   # [128, 1, 2, L]
    in_maps = []
    for b in range(N_CORES):
        xb = ip[b].astype(NP_BF16)                     # [C, 256, 256]
        xb = xb.reshape(C, 2, 128, L).transpose(2, 0, 1, 3)  # [128, C, 2, L]
        xb = np.concatenate([a, xb], axis=1)           # [128, C+1, 2, L]
        in_maps.append({"x": np.ascontiguousarray(xb)})
    return in_maps


def _unpack_out(results: list[dict[str, np.ndarray]]) -> np.ndarray:
    outs = []
    for b in range(N_CORES):
        ob = np.asarray(results[b]["out"])             # [128, C, 2, L] bf16
        ob = ob.transpose(1, 2, 0, 3).reshape(C, 256, 256).astype(np.float32)
        outs.append(ob)
    return np.stack(outs, axis=0)


def run(ip: np.ndarray, trace: bool = False):
    """Run the device kernel; returns (output, BassKernelResults)."""
    ip = np.asarray(ip)
    assert ip.shape == (N_CORES, C, 256, 256), ip.shape
    res = run_bass_kernel_spmd(
        _get_nc(), _make_in_maps(ip), core_ids=list(range(N_CORES)), trace=trace
    )
    return _unpack_out(res.results), res


def kernel(ip: np.ndarray) -> np.ndarray:
    out, _ = run(ip)
    return out


# revision 44
# speedup vs baseline: 1.0487x; 1.0487x over previous
"""2D DCT [8,32,256,256] on 8 TRN2 NeuronCores — raw Bass (no Tile).

Math: the reference's FFT-mirror trick is exactly the linear map
    dct1d(x)[k] = (1/L) * sum_m x[m] * cos(pi*k*(m+0.5)/L)
so with A[m,k] = cos(pi*k*(m+0.5)/L)/L the 2D DCT per [256,256] slice is
    out = A^T @ X @ A = (X^T A)^T A
i.e. two chained TensorEngine matmuls with NO transposes:
    V  = matmul(lhsT=X, rhs=A)   # V = X^T A   ([w, j] in PSUM)
    out= matmul(lhsT=V, rhs=A)   # V^T A = A^T X A  ([h', w'] in PSUM)

Sharding: fully data-parallel over batch — core b takes ip[b] (32
independent slices). bf16 staging in a [128, units, 2, 256] host layout
(contiguous per partition; unit 0 is the DCT matrix itself, units 1..32
the slices), f32 PSUM accumulation.

Raw-Bass engine plan (the Tile framework's entry/exit overhead and
per-instruction wait splitting cost ~8-10us here):
  SP (sync)  : one HWDGE ring for everything — graduated in-chunks
               (unit 0 rides with the first), then the out-chunks
               (issue stalls on copy sems; each DIRECT2D issue costs
               ~0.6-1.2us of sequencer time which conveniently paces
               the flood so other engines' instruction fetches aren't
               starved), final wait on out completions.
  PE         : 32 warm-up matmuls on garbage SBUF during the DMA head
               (HAM hits K=8/8 right as real data lands), then a
               software-pipelined stream S1(0..3), [S1(s), S2(s-4)],
               S2 tail; one pe_sem inc per 4-matmul stage.
  DVE / ACT  : whole-bank PSUM->SBUF evictions, one per stage,
               alternating engines per slice, each engine's stream
               sorted by pe_sem dependency (no head-of-line waits).
"""

import numpy as np

import concourse.bacc as bacc
import concourse.bass as bass
import concourse.mybir as mybir
from concourse.bass_utils import run_bass_kernel_spmd

N_CORES = 8
C = 32                    # slices per core (channel dim; batch is sharded)
L = 256                   # DCT length
BF16 = mybir.dt.bfloat16
F32 = mybir.dt.float32
NP_BF16 = mybir.dt.np(mybir.dt.bfloat16)

# In-chunks in UNITS of the staged tensor (unit 0 = DCT matrix,
# unit u = slice u-1), all on the sync HWDGE ring, graduated sizes.
IN_CHUNKS = [2, 1, 2, 4, 6, 6, 6, 6]
OUT_CHUNKS = [6, 6, 6, 6, 4, 2]           # slices per sync-ring out-DMA
TAIL_OUT = (30, 32)        # final slices: ACT issues one merged out-DMA
N_WARM = 30               # HAM warm-up matmuls during the DMA head
PS_RV = 4                 # vp PSUM ring depth (banks)
PS_RO = 3                 # op PSUM ring depth (banks)
VS_R = 6                  # vs SBUF ring depth
LOOKAHEAD = PS_RV         # S2(s) issues LOOKAHEAD stages after S1(s)


def _dct_matrix() -> np.ndarray:
    m = np.arange(L, dtype=np.float64)
    k = np.arange(L, dtype=np.float64)
    a = np.cos(np.pi * np.outer(m + 0.5, k) / L) / L
    a = a.astype(np.float32).astype(NP_BF16)
    # pack for SBUF: [p, ki, w] with row ki*128+p on partition p
    return np.ascontiguousarray(a.reshape(2, 128, L).transpose(1, 0, 2))


def _chunk_of_slice(s):
    """Chunk index covering slice s (= unit s+1)."""
    u = s + 1
    c0 = 0
    for ci, n in enumerate(IN_CHUNKS):
        if u < c0 + n:
            return ci
        c0 += n
    raise AssertionError


def _pe_schedule():
    order = []
    for s in range(C):
        order.append(("S1", s))
        if s >= LOOKAHEAD:
            order.append(("S2", s - LOOKAHEAD))
    for s in range(C - LOOKAHEAD, C):
        order.append(("S2", s))
    pe_count = {st: i + 1 for i, st in enumerate(order)}
    return order, pe_count


def _copy_plan(pe_count):
    """vs_copy(s) dep: S1(s); os_copy(s) dep: S2(s). Alternate engines by
    slice parity (the tail slices' os copies pinned to ACT so it can
    issue their out-DMAs inline); per-engine streams sorted by dep."""
    streams = {"dve": [], "act": []}
    for s in range(C):
        streams["dve" if s % 2 == 0 else "act"].append((pe_count[("S1", s)], "vs", s))
        streams["act" if s % 2 == 0 else "dve"].append((pe_count[("S2", s)], "os", s))
    pos = {}
    for eng, evs in streams.items():
        evs.sort()
        for i, (dep, kind, s) in enumerate(evs):
            pos[(kind, s)] = (eng, i + 1, dep)
    return streams, pos


def _build(sim: bool = False) -> bass.Bass:
    nc = bacc.Bacc()
    x = nc.declare_dram_parameter("x", [128, C + 1, 2, L], BF16, isOutput=False)
    out = nc.declare_dram_parameter("out", [128, C, 2, L], BF16, isOutput=True)

    order, pe_count = _pe_schedule()
    streams, pos = _copy_plan(pe_count)

    from contextlib import ExitStack

    ctx = ExitStack()
    with ctx:
        warm_sb = ctx.enter_context(nc.sbuf_tensor([128, 128], BF16))
        xs = ctx.enter_context(nc.sbuf_tensor([128, C + 1, 2, L], BF16))
        vs = ctx.enter_context(nc.sbuf_tensor([128, VS_R, 2, L], BF16))
        os_ = ctx.enter_context(nc.sbuf_tensor([128, C, 2, L], BF16))
        vp = ctx.enter_context(nc.psum_tensor([128, PS_RV, 2, L], F32))
        op = ctx.enter_context(nc.psum_tensor([128, PS_RO, 2, L], F32))
        warm_ps = ctx.enter_context(nc.psum_tensor([128, 128], F32))

        in_sems = [
            ctx.enter_context(nc.semaphore(f"in_sem{i}"))
            for i in range(len(IN_CHUNKS))
        ]
        pe_sem = ctx.enter_context(nc.semaphore("pe_sem"))
        dve_sem = ctx.enter_context(nc.semaphore("dve_sem"))
        act_sem = ctx.enter_context(nc.semaphore("act_sem"))
        out_sem = ctx.enter_context(nc.semaphore("out_sem"))
        warm_sem = ctx.enter_context(nc.semaphore("warm_sem"))
        sem_of = {"dve": dve_sem, "act": act_sem}

        block = ctx.enter_context(nc.Block(no_gpsimd_drain=True))

        @block.sync
        def _(eng):
            u0 = 0
            for ci, n in enumerate(IN_CHUNKS):
                eng.dma_start(
                    xs[:, u0 : u0 + n, :, :], x[:, u0 : u0 + n, :, :]
                ).then_inc(in_sems[ci], 16)
                u0 += n
            c0 = 0
            for n in OUT_CHUNKS:
                for eng_name in ("dve", "act"):
                    need = max(
                        (
                            pos[("os", s)][1]
                            for s in range(c0, c0 + n)
                            if pos[("os", s)][0] == eng_name
                        ),
                        default=0,
                    )
                    if need:
                        eng.wait_ge(sem_of[eng_name], need)
                eng.dma_start(
                    out[:, c0 : c0 + n, :, :], os_[:, c0 : c0 + n, :, :]
                ).then_inc(out_sem, 16)
                c0 += n
            eng.wait_ge(out_sem, 16 * (len(OUT_CHUNKS) + 1))

        @block.tensor
        def _(eng):
            if sim:
                # CoreSim rejects reads of uninitialized SBUF; on HW the
                # warm-up matmuls happily consume garbage.
                eng.wait_ge(warm_sem, 1)
            for _ in range(N_WARM):
                nc.tensor.matmul(
                    warm_ps[:], warm_sb[:], warm_sb[:], start=True, stop=True
                )
            seen_chunks = set()
            for kind, s in order:
                if kind == "S1":
                    ci = _chunk_of_slice(s)
                    if ci not in seen_chunks:
                        seen_chunks.add(ci)
                        eng.wait_ge(in_sems[ci], 16)
                    if s >= PS_RV:
                        # vp ring slot reuse: vs_copy(s-PS_RV) done
                        e, p, _ = pos[("vs", s - PS_RV)]
                        eng.wait_ge(sem_of[e], p)
                    r = s % PS_RV
                    for mi in range(2):
                        for ki in range(2):
                            mm = nc.tensor.matmul(
                                vp[:, r, mi, :],
                                xs[:, s + 1, ki, mi * 128 : (mi + 1) * 128],
                                xs[:, 0, ki, :],
                                start=(ki == 0),
                                stop=(ki == 1),
                            )
                    mm.then_inc(pe_sem, 1)
                else:
                    e, p, _ = pos[("vs", s)]          # vs(s) staged
                    eng.wait_ge(sem_of[e], p)
                    if s >= PS_RO:
                        # op ring slot reuse: os_copy(s-PS_RO) done
                        e, p, _ = pos[("os", s - PS_RO)]
                        eng.wait_ge(sem_of[e], p)
                    r = s % PS_RO
                    for ji in range(2):
                        for wi in range(2):
                            mm = nc.tensor.matmul(
                                op[:, r, ji, :],
                                vs[:, s % VS_R, wi, ji * 128 : (ji + 1) * 128],
                                xs[:, 0, wi, :],
                                start=(wi == 0),
                                stop=(wi == 1),
                            )
                    mm.then_inc(pe_sem, 1)

        def copy_stream(eng_name):
            def body(eng):
                copy = (
                    nc.vector.tensor_copy if eng_name == "dve" else nc.scalar.copy
                )
                if eng_name == "dve" and sim:
                    nc.vector.memset(warm_sb[:], 0.0).then_inc(warm_sem, 1)
                for dep, kind, s in streams[eng_name]:
                    eng.wait_ge(pe_sem, dep)
                    if kind == "vs":
                        copy(vs[:, s % VS_R, :, :], vp[:, s % PS_RV, :, :]).then_inc(
                            sem_of[eng_name], 1
                        )
                    else:
                        copy(os_[:, s, :, :], op[:, s % PS_RO, :, :]).then_inc(
                            sem_of[eng_name], 1
                        )
                if eng_name == "act":
                    # merged tail out-DMA: both final os copies done
                    # (own one by own-sem, the other engine's by sem)
                    lo, hi = TAIL_OUT
                    for s in range(lo, hi):
                        e, p, _ = pos[("os", s)]
                        eng.wait_ge(sem_of[e], p)
                    eng.dma_start(
                        out[:, lo:hi, :, :], os_[:, lo:hi, :, :]
                    ).then_inc(out_sem, 16)
            return body

        block.vector(copy_stream("dve"))
        block.scalar(copy_stream("act"))

    nc.compile()
    return nc


_NC_CACHE: bass.Bass | None = None


def _get_nc() -> bass.Bass:
    global _NC_CACHE
    if _NC_CACHE is None:
        _NC_CACHE = _build()
    return _NC_CACHE


def _make_in_maps(ip: np.ndarray) -> list[dict[str, np.ndarray]]:
    a = _dct_matrix()[:, None, :, :]                   # [128, 1, 2, L]
    in_maps = []
    for b in range(N_CORES):
        xb = ip[b].astype(NP_BF16)                     # [C, 256, 256]
        xb = xb.reshape(C, 2, 128, L).transpose(2, 0, 1, 3)  # [128, C, 2, L]
        xb = np.concatenate([a, xb], axis=1)           # [128, C+1, 2, L]
        in_maps.append({"x": np.ascontiguousarray(xb)})
    return in_maps


def _unpack_out(results: list[dict[str, np.ndarray]]) -> np.ndarray:
    outs = []
    for b in range(N_CORES):
        ob = np.asarray(results[b]["out"])             # [128, C, 2, L] bf16
        ob = ob.transpose(1, 2, 0, 3).reshape(C, 256, 256).astype(np.float32)
        outs.append(ob)
    return np.stack(outs, axis=0)


def run(ip: np.ndarray, trace: bool = False):
    """Run the device kernel; returns (output, BassKernelResults)."""
    ip = np.asarray(ip)
    assert ip.shape == (N_CORES, C, 256, 256), ip.shape
    res = run_bass_kernel_spmd(
        _get_nc(), _make_in_maps(ip), core_ids=list(range(N_CORES)), trace=trace
    )
    return _unpack_out(res.results), res


def kernel(ip: np.ndarray) -> np.ndarray:
    out, _ = run(ip)
    return out
